# revision 34
# baseline (speedup 1.0000x reference)
"""Trainium2 Bass kernel for nn_CountingAbstraction (sparse_attention).

Math (per batch b):
    cn  = l2_normalize(data[b], axis=-1)
    sim = relu(cn @ cn.T)                       # [N, N]
    counter_pre = sim @ [1 | fixed_v]           # rowsum + sim@posenc, [N, 513]
    counter = softplus(counter_pre @ W_exp + b_exp)
    out = [data | counter] @ W_merge

Device formulation (flash-attention-style fusion, never materializing sim):
    Wt = fixed_v @ W_exp[1:] + 1*W_exp[0]       # [N, M], folds rowsum+Dense
    z.T[m, q] = sum_k Wt[k, m] * relu(cn_k . cn_q)
    counter.T = softplus(z.T + b_exp)           # per-partition bias
    out[q, :] = rawq_q.T @ W_merge[:D] + counter.T.T @ W_merge[D:]

Everything that depends only on weights/constants is precomputed on host:
Wt (posenc @ W_exp[1:] + W_exp[0]) ships as fp8, and data ships twice —
l2-normalized fp8 (cn8, feeds the two big matmuls) and raw bf16 query rows
(rawq, feeds the merge). The two O(N^2 D) contractions (gram and z) run as
fp8 DoubleRow matmuls (2 contraction subtiles per instruction, 2x PE
throughput); the merge/output path stays bf16 so raw-data precision is kept.
softplus is computed as relu(z+b) [DVE] + ln(1 + exp(-|z+b|)) [ACT], which is
range-safe, and the merge matmuls of chunk ch-1 are emitted between the
k-loop and softplus of chunk ch so the in-order PE stream has work while ACT
runs the softplus chain.

Sharding: core c handles batch c//2, query rows half c%2 (2048 rows) against
all 4096 keys of that batch. Data-parallel, no collectives.
"""

import sys

for _p in ("/opt/trn_rl_repo",):
    if _p not in sys.path:
        sys.path.insert(0, _p)

import numpy as np
import ml_dtypes

import concourse.tile as tile
import concourse.mybir as mybir
from concourse import bacc
from concourse.bass import ts, ds
from concourse.bass_utils import run_bass_kernel_spmd

F32 = mybir.dt.float32
BF16 = mybir.dt.bfloat16
FP8 = mybir.dt.float8e4
AF = mybir.ActivationFunctionType
ALU = mybir.AluOpType
DR = mybir.MatmulPerfMode.DoubleRow
BF = ml_dtypes.bfloat16
F8 = ml_dtypes.float8_e4m3fn

B, N, D, M = 4, 4096, 512, 512
NCORES = 8
SWI_Z = True  # wt ships pre-interleaved; z matmuls use DoubleRowSwInterleave
SWI_G = False  # cn8 also ships interleaved (c8i); gram uses SwInterleave
MRAW8 = True   # raw-data merge term in fp8 DoubleRow (emulated rel 0.0056)
NQ = (B * N) // NCORES  # 2048 query rows per core


def _posenc(n, d):
    pos = np.arange(n, dtype=np.float32)[:, None]
    i = np.arange(d // 2, dtype=np.float32)[None, :]
    angle = pos / np.power(10000.0, 2.0 * i / d)
    pe = np.zeros((n, d), dtype=np.float32)
    pe[:, 0::2] = np.sin(angle)
    pe[:, 1::2] = np.cos(angle)
    return pe


def build_nc(nkeys=N, nq=NQ, qch=512, num_cores=NCORES, reps=1,
             act_front=0, sb_bufs=4, spread_start=6, spread_n=2,
             out_bf16=True, diag_depcut=False, diag=(), z_swi=SWI_Z,
             g_swi=SWI_G, z_lag=1, mraw8=True):
    """Build the SPMD Bass kernel (identical on every core).

    reps>1 wraps the whole body in a For_i hardware loop (same work each
    iteration, same in/out DRAM) — used only for timing, where it amortizes
    the fixed per-launch RPC overhead of the axon tunnel.
    """
    assert D % 256 == 0 and M % 128 == 0 and nkeys % 256 == 0
    assert nq % qch == 0 and qch % 128 == 0 and qch <= 512
    DP = D // 128       # contraction subtiles over feature dim
    MJ = M // 128       # output-column subtiles
    KB = nkeys // 128   # key blocks
    NCH = nq // qch     # query chunks

    nc = bacc.Bacc("TRN2", target_bir_lowering=False, debug=False,
                   num_devices=num_cores)
    c8 = nc.dram_tensor("c8", [D, nkeys], FP8, kind="ExternalInput").ap()
    c8i = (nc.dram_tensor("c8i", [128, (D // 256) * nkeys * 2], FP8,
                          kind="ExternalInput").ap() if g_swi else None)
    rq = nc.dram_tensor("rq", [D, nq], FP8 if mraw8 else BF16,
                        kind="ExternalInput").ap()
    wt8 = nc.dram_tensor("wt8", [128, KB * M], FP8, kind="ExternalInput").ap()
    wm1 = nc.dram_tensor("wm1", [D, M], FP8 if mraw8 else BF16,
                         kind="ExternalInput").ap()
    wm2 = nc.dram_tensor("wm2", [M, M], BF16, kind="ExternalInput").ap()
    bexp = nc.dram_tensor("bexp", [MJ, 128], F32, kind="ExternalInput").ap()
    odt = BF16 if out_bf16 else F32
    out = nc.dram_tensor("out", [nq, M], odt, kind="ExternalOutput").ap()

    qoff = 0  # query columns of c8 are rows [qoff, qoff+nq) — host slices rq

    with tile.TileContext(nc) as tc:
        with (
            tc.tile_pool(name="res", bufs=1) as res,
            tc.tile_pool(name="work", bufs=3) as work,
            tc.tile_pool(name="psg", bufs=3, space="PSUM") as psg,
            tc.tile_pool(name="psz", bufs=MJ, space="PSUM") as psz,
            tc.tile_pool(name="pso", bufs=1, space="PSUM") as pso,
        ):
          def _emit_body():
            # ---- residents (DMA priority order: first-gram data first) -----
            c8_sb = res.tile([128, DP, nkeys], FP8, tag="c8", name="c8_sb")
            c8i_sb = (res.tile([128, DP // 2, KB * 256], FP8, tag="c8i",
                               name="c8i_sb") if g_swi else None)
            if z_swi:
                wt_sb = res.tile([128, KB // 2, 2 * M], FP8, tag="wt",
                                 name="wt_sb")
            else:
                wt_sb = res.tile([128, KB, M], FP8, tag="wt", name="wt_sb")
            cgroups = [(0, 512), (512, 512), (1024, 1024), (2048, nkeys - 2048)]
            for c in range(DP):  # queries chunk 0 + first keys: gram kp0/kp1
                nc.sync.dma_start(c8_sb[:, c, ds(0, 512)], c8[ts(c, 128), ds(0, 512)])
            if g_swi:
                for d in range(DP // 2):  # first key blocks of the interleave
                    nc.sync.dma_start(c8i_sb[:, d, ds(0, 4 * 256)],
                                      c8i[:, ds(d * KB * 256, 4 * 256)])
            for g, (off, w) in enumerate(cgroups[1:], 1):
                for c in range(DP):
                    nc.sync.dma_start(c8_sb[:, c, ds(off, w)],
                                      c8[ts(c, 128), ds(off, w)])
                if g_swi:
                    for d in range(DP // 2):
                        o0, w0_ = (off // 128) * 256, (w // 128) * 256
                        if off == 512:
                            o0, w0_ = 4 * 256, 4 * 256
                        nc.sync.dma_start(
                            c8i_sb[:, d, ds(o0, w0_)],
                            c8i[:, ds(d * KB * 256 + o0, w0_)])
                wd = 2 * M if z_swi else M
                nc.sync.dma_start(wt_sb[:, ts(g - 1, KB // (4 * (2 if z_swi else 1))), :],
                                  wt8[:, ts(g - 1, (KB // 4) * M)]
                                  .rearrange("p (k m) -> p k m", m=wd))
            nc.sync.dma_start(wt_sb[:, ts(3, KB // (4 * (2 if z_swi else 1))), :],
                              wt8[:, ts(3, (KB // 4) * M)]
                              .rearrange("p (k m) -> p k m", m=2 * M if z_swi else M))
            bexp_sb = res.tile([128, MJ], F32, tag="bexp", name="bexp_sb")
            nc.sync.dma_start(bexp_sb[:], bexp.rearrange("c p -> p c"))
            rawq = res.tile([128, DP, nq], FP8 if mraw8 else BF16,
                            tag="rawq", name="rawq")
            for c in range(DP):
                nc.sync.dma_start(rawq[:, c, :], rq[ts(c, 128), :])
            wm1_sb = res.tile([128, DP, M], FP8 if mraw8 else BF16,
                              tag="wm1", name="wm1_sb")
            nc.sync.dma_start(wm1_sb[:], wm1.rearrange("(c p) m -> p c m", p=128))
            wm2_sb = res.tile([128, MJ, M], BF16, tag="wm2", name="wm2_sb")
            nc.sync.dma_start(wm2_sb[:], wm2.rearrange("(c p) m -> p c m", p=128))

            # ---- fused sim / counter / merge -------------------------------
            # gram(ki): sim key-block ki vs this chunk's queries, fp8
            # DoubleRow over dp pairs; relu lands fp8 in slot j of a paired
            # sb tile; z consumes pairs (lagged one pair so the PE stream
            # never waits on the relu of the tile it is about to consume).
            # relu engine plan: the first act_front pairs of each chunk run
            # both relus on ACT (ACT reads PSUM ~1.8x faster than DVE and is
            # otherwise idle at chunk start after the pz-evacuation copies);
            # later pairs split slot0->DVE / slot1->ACT so both engines pace
            # under the PE. The previous chunk's softplus chain is drip-fed
            # into ACT one op per pair via the worklist so it never forms a
            # backlog that stalls the z pipeline.

            def gram_part(ch, ki, sb2, j, on_act):
                if "nogram" in diag:
                    return
                ps = psg.tile([128, qch], F32, tag="ps", name="ps")
                qc = ds(qoff + ch * qch, qch)
                for d in range(DP // 2):
                    if g_swi:
                        lhsT = (c8i_sb[:, d, ds(ki * 256, 256)]
                                .rearrange("p (j c) -> p j c", j=2))
                        pm = mybir.MatmulPerfMode.DoubleRowSwInterleave
                    else:
                        lhsT = c8_sb[:, 2 * d:2 * d + 2, ts(ki, 128)]
                        pm = DR
                    nc.tensor.matmul(ps[:], lhsT,
                                     c8_sb[:, 2 * d:2 * d + 2, qc],
                                     start=(d == 0), stop=(d == DP // 2 - 1),
                                     perf_mode=pm)
                if "norelu" in diag:
                    return
                if on_act:
                    nc.scalar.activation(sb2[:, j, :], ps[:], AF.Relu)
                else:
                    nc.vector.tensor_scalar(sb2[:, j, :], ps[:], 0.0, None,
                                            ALU.max)

            depcut = diag_depcut or "norelu" in diag or "nogram" in diag
            cb2 = None
            if depcut:
                cb2 = res.tile([128, 2, qch], FP8, tag="cb2", name="cb2")
                nc.vector.memset(cb2[:], 0.25)

            def z_part(kp, sb2, pz):
                if "noz" in diag:
                    return
                if depcut:
                    sb2 = cb2
                for mj in range(MJ):
                    if z_swi:
                        # wt ships host-interleaved (A/B key-subtile pairs,
                        # columns reversed) so the 256-col stationary load is
                        # a contiguous read instead of DoubleRow's strided
                        # interleave fetch.
                        lhsT = (wt_sb[:, kp, ds(mj * 256, 256)]
                                .rearrange("p (j c) -> p j c", j=2))
                        pm = mybir.MatmulPerfMode.DoubleRowSwInterleave
                    else:
                        lhsT = wt_sb[:, 2 * kp:2 * kp + 2, ts(mj, 128)]
                        pm = DR
                    nc.tensor.matmul(pz[mj][:], lhsT, sb2[:, 0:2, :],
                                     start=(kp == 0), stop=(kp == KB // 2 - 1),
                                     perf_mode=pm)

            def emit_merge(ch, cts, spread=False):
                if "nomerge" in diag:
                    return
                # spread=True (last chunk): all raw-data matmuls first across
                # po banks borrowed from the idle gram pool, so the PE has
                # ct-independent work while ACT finishes the final softplus.
                qss = range(qch // 128)
                pos = {}
                for qs in qss:
                    pool = (psg if spread and qs < 3 else pso)
                    po = pool.tile([128, M], F32, tag="ps" if pool is psg else "po",
                                   name=f"po{qs}")
                    pos[qs] = po
                    if mraw8:
                        for d in range(DP // 2):
                            nc.tensor.matmul(
                                po[:],
                                rawq[:, 2 * d:2 * d + 2,
                                     ds(ch * qch + qs * 128, 128)],
                                wm1_sb[:, 2 * d:2 * d + 2, :],
                                start=(d == 0), stop=False, perf_mode=DR)
                    else:
                        for dp in range(DP):
                            nc.tensor.matmul(
                                po[:],
                                rawq[:, dp, ds(ch * qch + qs * 128, 128)],
                                wm1_sb[:, dp, :],
                                start=(dp == 0), stop=False)
                    if not spread:
                        _merge_ct(ch, qs, po, cts)
                if spread:
                    for qs in qss:
                        _merge_ct(ch, qs, pos[qs], cts)

            def _merge_ct(ch, qs, po, cts):
                for mj in range(MJ):
                    nc.tensor.matmul(po[:], cts[mj][:, ts(qs, 128)],
                                     wm2_sb[:, mj, :],
                                     start=False, stop=(mj == MJ - 1))
                ob = work.tile([128, M], odt, tag="ob", bufs=2, name="ob")
                nc.vector.tensor_copy(ob[:], po[:])
                nc.sync.dma_start(out[ds(ch * qch + qs * 128, 128), :], ob[:])

            ct_const = None

            def emit_softplus(pz):
                nonlocal ct_const
                if "nochain" in diag:
                    if ct_const is None:
                        ct_const = [res.tile([128, qch], BF16, tag=f"ctc{m}",
                                             name=f"ctc{m}") for m in range(MJ)]
                        for t in ct_const:
                            nc.vector.memset(t[:], 0.5)
                    return [], ct_const
                # Evacuate z from PSUM to SBUF immediately (split DVE/ACT so
                # the next chunk's z matmuls get their PSUM banks back after
                # one fast read each), then t1 = relu(z+b) on DVE. The rest
                # of softplus — ln(1 + exp(-|zb|)) on ACT and the final add
                # on DVE — is returned as a worklist of closures that
                # run_chunk drip-feeds between gram pairs of the NEXT chunk,
                # so the in-order ACT/DVE queues never hold a softplus
                # backlog in front of the relus the z pipeline needs.
                zbs, t1s, cts = [], [], []
                for mj in range(MJ):
                    zb = work.tile([128, qch], F32, tag="zb", bufs=8, name="zb")
                    if mj % 2 == 0:
                        nc.vector.tensor_copy(zb[:], pz[mj][:])
                    else:
                        nc.scalar.activation(zb[:], pz[mj][:], AF.Copy)
                    zbs.append(zb)
                for mj in range(MJ):
                    bmj = bexp_sb[:, mj:mj + 1]
                    t1 = work.tile([128, qch], F32, tag="t1", bufs=8, name="t1")
                    nc.vector.tensor_scalar(t1[:], zbs[mj][:], bmj, 0.0,
                                            ALU.add, ALU.max)
                    t1s.append(t1)
                t2s = [work.tile([128, qch], F32, tag="t2", bufs=4, name="t2")
                       for _ in range(MJ)]
                t3s = [work.tile([128, qch], F32, tag="t3", bufs=4, name="t3")
                       for _ in range(MJ)]
                t4s = [work.tile([128, qch], F32, tag="t4", bufs=4, name="t4")
                       for _ in range(MJ)]
                cts = [work.tile([128, qch], BF16, tag="ct", bufs=8, name="ct")
                       for _ in range(MJ)]
                wl = []
                for mj in range(MJ):
                    bmj = bexp_sb[:, mj:mj + 1]
                    wl.append(lambda mj=mj, bmj=bmj: nc.scalar.activation(
                        t2s[mj][:], zbs[mj][:], AF.Abs, bias=bmj))
                for mj in range(MJ):
                    wl.append(lambda mj=mj: nc.scalar.activation(
                        t3s[mj][:], t2s[mj][:], AF.Exp, scale=-1.0))
                for mj in range(MJ):
                    wl.append(lambda mj=mj: nc.scalar.activation(
                        t4s[mj][:], t3s[mj][:], AF.Ln, bias=1.0))
                for mj in range(MJ):
                    wl.append(lambda mj=mj: nc.vector.tensor_add(
                        cts[mj][:], t1s[mj][:], t4s[mj][:]))
                return wl, cts

            def run_chunk(ch, pz, wl=()):
                wl = list(wl)
                pending = []
                for kp in range(KB // 2):
                    sb2 = work.tile([128, 2, qch], FP8, tag="sb", bufs=sb_bufs,
                                    name="sb2")
                    front = kp < act_front
                    gram_part(ch, 2 * kp, sb2, 0, front)
                    gram_part(ch, 2 * kp + 1, sb2, 1, True)
                    pending.append((kp, sb2))
                    if len(pending) > z_lag:
                        z_part(*pending.pop(0), pz)
                    if kp >= spread_start:
                        for _ in range(spread_n):
                            if wl:
                                wl.pop(0)()
                while pending:
                    z_part(*pending.pop(0), pz)
                while wl:
                    wl.pop(0)()

            def alloc_pz():
                return [psz.tile([128, qch], F32, tag="pz", name=f"pz{mj}")
                        for mj in range(MJ)]

            pz = alloc_pz()
            run_chunk(0, pz)
            wl, cts_prev = emit_softplus(pz)
            for ch in range(1, NCH):
                pz = alloc_pz()
                run_chunk(ch, pz, wl)
                emit_merge(ch - 1, cts_prev)
                wl, cts_prev = emit_softplus(pz)
            for fn in wl:
                fn()
            emit_merge(NCH - 1, cts_prev, spread=True)

          if reps == 1:
              _emit_body()
          else:
              # branch-prefetch hints: the body is ~1700 instructions, far
              # over one IRAM block, so an unhinted back-edge I$-misses
              # (~3-4 us DMA fetch) on every engine each iteration.
              ET = mybir.EngineType
              with tc.For_i(0, reps, 1, hint_engines=(ET.PE, ET.DVE,
                                                      ET.Activation, ET.SP,
                                                      ET.Pool)):
                  _emit_body()

    nc.compile()
    return nc


def make_in_maps(data, W_exp, b_exp, W_merge, num_cores=NCORES):
    """Host prep: normalize/transpose/cast inputs into per-core input maps."""
    data = np.asarray(data, dtype=np.float32)
    W_exp = np.asarray(W_exp, dtype=np.float32)
    b_exp = np.asarray(b_exp, dtype=np.float32)
    W_merge = np.asarray(W_merge, dtype=np.float32)

    dataT = np.ascontiguousarray(data.transpose(0, 2, 1))  # [B, D, N] f32
    rn = 1.0 / np.sqrt(np.maximum((dataT * dataT).sum(axis=1), 1e-12))
    cn8 = [np.ascontiguousarray((dataT[b] * rn[b][None, :]).astype(F8))
           for b in range(B)]

    def pack_c8i(cn):
        # raw[p, d, ki, 2t+j] = cn[(2d+j)*128 + p, ki*128 + (127-t)]
        DPh, KBk = D // 256, N // 128
        C = np.asarray(cn).reshape(DPh, 2, 128, KBk, 128)
        C = C[..., ::-1]
        C = C.transpose(2, 0, 3, 4, 1)
        return np.ascontiguousarray(C.reshape(128, DPh * KBk * 256))
    rq_c = [np.ascontiguousarray(dataT[b].astype(F8 if MRAW8 else BF))
            for b in range(B)]

    Wt = _posenc(N, D) @ W_exp[1:] + W_exp[0]               # [N, M]
    KB = N // 128

    def pack_wt(W):
        if SWI_Z:
            # DoubleRowSwInterleave layout: per (kp, mj) a 256-col block
            # raw[p, 2t+j] = W[(2kp+j)*128 + p, mj*128 + (127-t)]
            C = W.reshape(KB // 2, 2, 128, M // 128, 128)  # [kp, j, p, mj, c]
            C = C[..., ::-1]                               # c -> t
            C = C.transpose(2, 0, 3, 4, 1)                 # [p, kp, mj, t, j]
            return np.ascontiguousarray(
                C.reshape(128, KB * M)).astype(F8)
        return np.ascontiguousarray(
            W.reshape(KB, 128, M).transpose(1, 0, 2).reshape(128, KB * M)
        ).astype(F8)

    # rotate key columns so this core's query rows are always keys [0:NQ];
    # Wt is rotated identically (the k-sum is permutation-invariant when
    # Wt rows follow their keys).
    wt8 = [pack_wt(Wt), pack_wt(np.roll(Wt, -NQ, axis=0))]
    cn8_rot = [np.ascontiguousarray(np.roll(a, -NQ, axis=1)) for a in cn8]
    c8i = [pack_c8i(a) for a in cn8] if SWI_G else [None] * B
    c8i_rot = [pack_c8i(a) for a in cn8_rot] if SWI_G else [None] * B
    wm1_c = W_merge[:D].astype(F8 if MRAW8 else BF)
    wm2_bf = W_merge[D:].astype(BF)
    bexp_r = np.ascontiguousarray(b_exp.reshape(M // 128, 128))

    in_maps = []
    for c in range(num_cores):
        b, h = c // 2, c % 2
        m = {
            "c8": cn8[b] if h == 0 else cn8_rot[b],
            "rq": np.ascontiguousarray(rq_c[b][:, h * NQ:(h + 1) * NQ]),
            "wt8": wt8[h],
            "wm1": wm1_c,
            "wm2": wm2_bf,
            "bexp": bexp_r,
        }
        if SWI_G:
            m["c8i"] = c8i[b] if h == 0 else c8i_rot[b]
        in_maps.append(m)
    return in_maps


_NC_CACHE = {}


def get_nc():
    if "full" not in _NC_CACHE:
        _NC_CACHE["full"] = build_nc()
    return _NC_CACHE["full"]


def kernel(data, W_exp, b_exp, W_merge):
    nc = get_nc()
    in_maps = make_in_maps(data, W_exp, b_exp, W_merge)
    res = run_bass_kernel_spmd(nc, in_maps, core_ids=list(range(NCORES)))
    out = np.empty((B, N, M), dtype=np.float32)
    for c in range(NCORES):
        b, h = c // 2, c % 2
        out[b, h * NQ:(h + 1) * NQ] = res.results[c]["out"].astype(np.float32)
    return out


# revision 35
# speedup vs baseline: 1.1183x; 1.1183x over previous
"""Trainium2 Bass kernel for nn_CountingAbstraction (sparse_attention).

Math (per batch b):
    cn  = l2_normalize(data[b], axis=-1)
    sim = relu(cn @ cn.T)                       # [N, N]
    counter_pre = sim @ [1 | fixed_v]           # rowsum + sim@posenc, [N, 513]
    counter = softplus(counter_pre @ W_exp + b_exp)
    out = [data | counter] @ W_merge

Device formulation (flash-attention-style fusion, never materializing sim):
    Wt = fixed_v @ W_exp[1:] + 1*W_exp[0]       # [N, M], folds rowsum+Dense
    z.T[m, q] = sum_k Wt[k, m] * relu(cn_k . cn_q)
    counter.T = softplus(z.T + b_exp)           # per-partition bias
    out[q, :] = rawq_q.T @ W_merge[:D] + counter.T.T @ W_merge[D:]

Everything that depends only on weights/constants is precomputed on host:
Wt (posenc @ W_exp[1:] + W_exp[0]) ships as fp8, and data ships twice —
l2-normalized fp8 (cn8, feeds the two big matmuls) and raw bf16 query rows
(rawq, feeds the merge). The two O(N^2 D) contractions (gram and z) run as
fp8 DoubleRow matmuls (2 contraction subtiles per instruction, 2x PE
throughput); the merge/output path stays bf16 so raw-data precision is kept.
softplus is computed as relu(z+b) [DVE] + ln(1 + exp(-|z+b|)) [ACT], which is
range-safe, and the merge matmuls of chunk ch-1 are emitted between the
k-loop and softplus of chunk ch so the in-order PE stream has work while ACT
runs the softplus chain.

Sharding: core c handles batch c//2, query rows half c%2 (2048 rows) against
all 4096 keys of that batch. Data-parallel, no collectives.
"""

import sys

for _p in ("/opt/trn_rl_repo",):
    if _p not in sys.path:
        sys.path.insert(0, _p)

import numpy as np
import ml_dtypes

import concourse.tile as tile
import concourse.mybir as mybir
from concourse import bacc
from concourse.bass import ts, ds
from concourse.bass_utils import run_bass_kernel_spmd

F32 = mybir.dt.float32
BF16 = mybir.dt.bfloat16
FP8 = mybir.dt.float8e4
AF = mybir.ActivationFunctionType
ALU = mybir.AluOpType
DR = mybir.MatmulPerfMode.DoubleRow
BF = ml_dtypes.bfloat16
F8 = ml_dtypes.float8_e4m3fn

B, N, D, M = 4, 4096, 512, 512
NCORES = 8
SWI_Z = True  # wt ships pre-interleaved; z matmuls use DoubleRowSwInterleave
SWI_G = False  # cn8 also ships interleaved (c8i); gram uses SwInterleave
MRAW8 = True   # raw-data merge term in fp8 DoubleRow (emulated rel 0.0056)
NQ = (B * N) // NCORES  # 2048 query rows per core


def _posenc(n, d):
    pos = np.arange(n, dtype=np.float32)[:, None]
    i = np.arange(d // 2, dtype=np.float32)[None, :]
    angle = pos / np.power(10000.0, 2.0 * i / d)
    pe = np.zeros((n, d), dtype=np.float32)
    pe[:, 0::2] = np.sin(angle)
    pe[:, 1::2] = np.cos(angle)
    return pe


def build_nc(nkeys=N, nq=NQ, qch=512, num_cores=NCORES, reps=1,
             act_front=0, sb_bufs=4, spread_start=6, spread_n=2,
             out_bf16=True, diag_depcut=False, diag=(), z_swi=SWI_Z,
             g_swi=SWI_G, z_lag=1, mraw8=True, unroll=1):
    """Build the SPMD Bass kernel (identical on every core).

    reps>1 wraps the whole body in a For_i hardware loop (same work each
    iteration, same in/out DRAM) — used only for timing, where it amortizes
    the fixed per-launch RPC overhead of the axon tunnel.
    """
    assert D % 256 == 0 and M % 128 == 0 and nkeys % 256 == 0
    assert nq % qch == 0 and qch % 128 == 0 and qch <= 512
    DP = D // 128       # contraction subtiles over feature dim
    MJ = M // 128       # output-column subtiles
    KB = nkeys // 128   # key blocks
    NCH = nq // qch     # query chunks

    nc = bacc.Bacc("TRN2", target_bir_lowering=False, debug=False,
                   num_devices=num_cores)
    c8 = nc.dram_tensor("c8", [D, nkeys], FP8, kind="ExternalInput").ap()
    c8i = (nc.dram_tensor("c8i", [128, (D // 256) * nkeys * 2], FP8,
                          kind="ExternalInput").ap() if g_swi else None)
    rq = nc.dram_tensor("rq", [D, nq], FP8 if mraw8 else BF16,
                        kind="ExternalInput").ap()
    wt8 = nc.dram_tensor("wt8", [128, KB * M], FP8, kind="ExternalInput").ap()
    wm1 = nc.dram_tensor("wm1", [D, M], FP8 if mraw8 else BF16,
                         kind="ExternalInput").ap()
    wm2 = nc.dram_tensor("wm2", [M, M], BF16, kind="ExternalInput").ap()
    bexp = nc.dram_tensor("bexp", [MJ, 128], F32, kind="ExternalInput").ap()
    odt = BF16 if out_bf16 else F32
    out = nc.dram_tensor("out", [nq, M], odt, kind="ExternalOutput").ap()

    qoff = 0  # query columns of c8 are rows [qoff, qoff+nq) — host slices rq

    with tile.TileContext(nc) as tc:
        with (
            tc.tile_pool(name="res", bufs=1) as res,
            tc.tile_pool(name="work", bufs=3) as work,
            tc.tile_pool(name="psg", bufs=3, space="PSUM") as psg,
            tc.tile_pool(name="psz", bufs=MJ, space="PSUM") as psz,
            tc.tile_pool(name="pso", bufs=1, space="PSUM") as pso,
        ):
          def _emit_body():
            # ---- residents (DMA priority order: first-gram data first) -----
            c8_sb = res.tile([128, DP, nkeys], FP8, tag="c8", name="c8_sb")
            c8i_sb = (res.tile([128, DP // 2, KB * 256], FP8, tag="c8i",
                               name="c8i_sb") if g_swi else None)
            if z_swi:
                wt_sb = res.tile([128, KB // 2, 2 * M], FP8, tag="wt",
                                 name="wt_sb")
            else:
                wt_sb = res.tile([128, KB, M], FP8, tag="wt", name="wt_sb")
            cgroups = [(0, 512), (512, 512), (1024, 1024), (2048, nkeys - 2048)]
            for c in range(DP):  # queries chunk 0 + first keys: gram kp0/kp1
                nc.sync.dma_start(c8_sb[:, c, ds(0, 512)], c8[ts(c, 128), ds(0, 512)])
            if g_swi:
                for d in range(DP // 2):  # first key blocks of the interleave
                    nc.sync.dma_start(c8i_sb[:, d, ds(0, 4 * 256)],
                                      c8i[:, ds(d * KB * 256, 4 * 256)])
            for g, (off, w) in enumerate(cgroups[1:], 1):
                for c in range(DP):
                    nc.sync.dma_start(c8_sb[:, c, ds(off, w)],
                                      c8[ts(c, 128), ds(off, w)])
                if g_swi:
                    for d in range(DP // 2):
                        o0, w0_ = (off // 128) * 256, (w // 128) * 256
                        if off == 512:
                            o0, w0_ = 4 * 256, 4 * 256
                        nc.sync.dma_start(
                            c8i_sb[:, d, ds(o0, w0_)],
                            c8i[:, ds(d * KB * 256 + o0, w0_)])
                wd = 2 * M if z_swi else M
                nc.sync.dma_start(wt_sb[:, ts(g - 1, KB // (4 * (2 if z_swi else 1))), :],
                                  wt8[:, ts(g - 1, (KB // 4) * M)]
                                  .rearrange("p (k m) -> p k m", m=wd))
            nc.sync.dma_start(wt_sb[:, ts(3, KB // (4 * (2 if z_swi else 1))), :],
                              wt8[:, ts(3, (KB // 4) * M)]
                              .rearrange("p (k m) -> p k m", m=2 * M if z_swi else M))
            bexp_sb = res.tile([128, MJ], F32, tag="bexp", name="bexp_sb")
            nc.sync.dma_start(bexp_sb[:], bexp.rearrange("c p -> p c"))
            rawq = res.tile([128, DP, nq], FP8 if mraw8 else BF16,
                            tag="rawq", name="rawq")
            for c in range(DP):
                nc.sync.dma_start(rawq[:, c, :], rq[ts(c, 128), :])
            wm1_sb = res.tile([128, DP, M], FP8 if mraw8 else BF16,
                              tag="wm1", name="wm1_sb")
            nc.sync.dma_start(wm1_sb[:], wm1.rearrange("(c p) m -> p c m", p=128))
            wm2_sb = res.tile([128, MJ, M], BF16, tag="wm2", name="wm2_sb")
            nc.sync.dma_start(wm2_sb[:], wm2.rearrange("(c p) m -> p c m", p=128))

            # ---- fused sim / counter / merge -------------------------------
            # gram(ki): sim key-block ki vs this chunk's queries, fp8
            # DoubleRow over dp pairs; relu lands fp8 in slot j of a paired
            # sb tile; z consumes pairs (lagged one pair so the PE stream
            # never waits on the relu of the tile it is about to consume).
            # relu engine plan: the first act_front pairs of each chunk run
            # both relus on ACT (ACT reads PSUM ~1.8x faster than DVE and is
            # otherwise idle at chunk start after the pz-evacuation copies);
            # later pairs split slot0->DVE / slot1->ACT so both engines pace
            # under the PE. The previous chunk's softplus chain is drip-fed
            # into ACT one op per pair via the worklist so it never forms a
            # backlog that stalls the z pipeline.

            def gram_part(ch, ki, sb2, j, on_act):
                if "nogram" in diag:
                    return
                ps = psg.tile([128, qch], F32, tag="ps", name="ps")
                qc = ds(qoff + ch * qch, qch)
                for d in range(DP // 2):
                    if g_swi:
                        lhsT = (c8i_sb[:, d, ds(ki * 256, 256)]
                                .rearrange("p (j c) -> p j c", j=2))
                        pm = mybir.MatmulPerfMode.DoubleRowSwInterleave
                    else:
                        lhsT = c8_sb[:, 2 * d:2 * d + 2, ts(ki, 128)]
                        pm = DR
                    nc.tensor.matmul(ps[:], lhsT,
                                     c8_sb[:, 2 * d:2 * d + 2, qc],
                                     start=(d == 0), stop=(d == DP // 2 - 1),
                                     perf_mode=pm)
                if "norelu" in diag:
                    return
                if on_act:
                    nc.scalar.activation(sb2[:, j, :], ps[:], AF.Relu)
                else:
                    nc.vector.tensor_scalar(sb2[:, j, :], ps[:], 0.0, None,
                                            ALU.max)

            depcut = diag_depcut or "norelu" in diag or "nogram" in diag
            cb2 = None
            if depcut:
                cb2 = res.tile([128, 2, qch], FP8, tag="cb2", name="cb2")
                nc.vector.memset(cb2[:], 0.25)

            def z_part(kp, sb2, pz):
                if "noz" in diag:
                    return
                if depcut:
                    sb2 = cb2
                for mj in range(MJ):
                    if z_swi:
                        # wt ships host-interleaved (A/B key-subtile pairs,
                        # columns reversed) so the 256-col stationary load is
                        # a contiguous read instead of DoubleRow's strided
                        # interleave fetch.
                        lhsT = (wt_sb[:, kp, ds(mj * 256, 256)]
                                .rearrange("p (j c) -> p j c", j=2))
                        pm = mybir.MatmulPerfMode.DoubleRowSwInterleave
                    else:
                        lhsT = wt_sb[:, 2 * kp:2 * kp + 2, ts(mj, 128)]
                        pm = DR
                    nc.tensor.matmul(pz[mj][:], lhsT, sb2[:, 0:2, :],
                                     start=(kp == 0), stop=(kp == KB // 2 - 1),
                                     perf_mode=pm)

            def emit_merge(ch, cts, spread=False):
                if "nomerge" in diag:
                    return
                # spread=True (last chunk): all raw-data matmuls first across
                # po banks borrowed from the idle gram pool, so the PE has
                # ct-independent work while ACT finishes the final softplus.
                qss = range(qch // 128)
                pos = {}
                for qs in qss:
                    pool = (psg if spread and qs < 3 else pso)
                    po = pool.tile([128, M], F32, tag="ps" if pool is psg else "po",
                                   name=f"po{qs}")
                    pos[qs] = po
                    if mraw8:
                        for d in range(DP // 2):
                            nc.tensor.matmul(
                                po[:],
                                rawq[:, 2 * d:2 * d + 2,
                                     ds(ch * qch + qs * 128, 128)],
                                wm1_sb[:, 2 * d:2 * d + 2, :],
                                start=(d == 0), stop=False, perf_mode=DR)
                    else:
                        for dp in range(DP):
                            nc.tensor.matmul(
                                po[:],
                                rawq[:, dp, ds(ch * qch + qs * 128, 128)],
                                wm1_sb[:, dp, :],
                                start=(dp == 0), stop=False)
                    if not spread:
                        _merge_ct(ch, qs, po, cts)
                if spread:
                    for qs in qss:
                        _merge_ct(ch, qs, pos[qs], cts)

            def _merge_ct(ch, qs, po, cts):
                for mj in range(MJ):
                    nc.tensor.matmul(po[:], cts[mj][:, ts(qs, 128)],
                                     wm2_sb[:, mj, :],
                                     start=False, stop=(mj == MJ - 1))
                ob = work.tile([128, M], odt, tag="ob", bufs=2, name="ob")
                nc.vector.tensor_copy(ob[:], po[:])
                nc.sync.dma_start(out[ds(ch * qch + qs * 128, 128), :], ob[:])

            ct_const = None

            def emit_softplus(pz):
                nonlocal ct_const
                if "nochain" in diag:
                    if ct_const is None:
                        ct_const = [res.tile([128, qch], BF16, tag=f"ctc{m}",
                                             name=f"ctc{m}") for m in range(MJ)]
                        for t in ct_const:
                            nc.vector.memset(t[:], 0.5)
                    return [], ct_const
                # Evacuate z from PSUM to SBUF immediately (split DVE/ACT so
                # the next chunk's z matmuls get their PSUM banks back after
                # one fast read each), then t1 = relu(z+b) on DVE. The rest
                # of softplus — ln(1 + exp(-|zb|)) on ACT and the final add
                # on DVE — is returned as a worklist of closures that
                # run_chunk drip-feeds between gram pairs of the NEXT chunk,
                # so the in-order ACT/DVE queues never hold a softplus
                # backlog in front of the relus the z pipeline needs.
                zbs, t1s, cts = [], [], []
                for mj in range(MJ):
                    zb = work.tile([128, qch], F32, tag="zb", bufs=8, name="zb")
                    if mj % 2 == 0:
                        nc.vector.tensor_copy(zb[:], pz[mj][:])
                    else:
                        nc.scalar.activation(zb[:], pz[mj][:], AF.Copy)
                    zbs.append(zb)
                for mj in range(MJ):
                    bmj = bexp_sb[:, mj:mj + 1]
                    t1 = work.tile([128, qch], F32, tag="t1", bufs=8, name="t1")
                    nc.vector.tensor_scalar(t1[:], zbs[mj][:], bmj, 0.0,
                                            ALU.add, ALU.max)
                    t1s.append(t1)
                t2s = [work.tile([128, qch], F32, tag="t2", bufs=4, name="t2")
                       for _ in range(MJ)]
                t3s = [work.tile([128, qch], F32, tag="t3", bufs=4, name="t3")
                       for _ in range(MJ)]
                t4s = [work.tile([128, qch], F32, tag="t4", bufs=4, name="t4")
                       for _ in range(MJ)]
                cts = [work.tile([128, qch], BF16, tag="ct", bufs=8, name="ct")
                       for _ in range(MJ)]
                wl = []
                for mj in range(MJ):
                    bmj = bexp_sb[:, mj:mj + 1]
                    wl.append(lambda mj=mj, bmj=bmj: nc.scalar.activation(
                        t2s[mj][:], zbs[mj][:], AF.Abs, bias=bmj))
                for mj in range(MJ):
                    wl.append(lambda mj=mj: nc.scalar.activation(
                        t3s[mj][:], t2s[mj][:], AF.Exp, scale=-1.0))
                for mj in range(MJ):
                    wl.append(lambda mj=mj: nc.scalar.activation(
                        t4s[mj][:], t3s[mj][:], AF.Ln, bias=1.0))
                for mj in range(MJ):
                    wl.append(lambda mj=mj: nc.vector.tensor_add(
                        cts[mj][:], t1s[mj][:], t4s[mj][:]))
                return wl, cts

            def run_chunk(ch, pz, wl=()):
                wl = list(wl)
                pending = []
                for kp in range(KB // 2):
                    sb2 = work.tile([128, 2, qch], FP8, tag="sb", bufs=sb_bufs,
                                    name="sb2")
                    front = kp < act_front
                    gram_part(ch, 2 * kp, sb2, 0, front)
                    gram_part(ch, 2 * kp + 1, sb2, 1, True)
                    pending.append((kp, sb2))
                    if len(pending) > z_lag:
                        z_part(*pending.pop(0), pz)
                    if kp >= spread_start:
                        for _ in range(spread_n):
                            if wl:
                                wl.pop(0)()
                while pending:
                    z_part(*pending.pop(0), pz)
                while wl:
                    wl.pop(0)()

            def alloc_pz():
                return [psz.tile([128, qch], F32, tag="pz", name=f"pz{mj}")
                        for mj in range(MJ)]

            pz = alloc_pz()
            run_chunk(0, pz)
            wl, cts_prev = emit_softplus(pz)
            for ch in range(1, NCH):
                pz = alloc_pz()
                run_chunk(ch, pz, wl)
                emit_merge(ch - 1, cts_prev)
                wl, cts_prev = emit_softplus(pz)
            for fn in wl:
                fn()
            emit_merge(NCH - 1, cts_prev, spread=True)

          if reps == 1:
              for _ in range(unroll):
                  _emit_body()
          else:
              # branch-prefetch hints: the body is ~1700 instructions, far
              # over one IRAM block, so an unhinted back-edge I$-misses
              # (~3-4 us DMA fetch) on every engine each iteration.
              ET = mybir.EngineType
              with tc.For_i(0, reps, 1, hint_engines=(ET.PE, ET.DVE,
                                                      ET.Activation, ET.SP,
                                                      ET.Pool)):
                  for _ in range(unroll):
                      _emit_body()

    nc.compile()
    return nc


def make_in_maps(data, W_exp, b_exp, W_merge, num_cores=NCORES):
    """Host prep: normalize/transpose/cast inputs into per-core input maps."""
    data = np.asarray(data, dtype=np.float32)
    W_exp = np.asarray(W_exp, dtype=np.float32)
    b_exp = np.asarray(b_exp, dtype=np.float32)
    W_merge = np.asarray(W_merge, dtype=np.float32)

    dataT = np.ascontiguousarray(data.transpose(0, 2, 1))  # [B, D, N] f32
    rn = 1.0 / np.sqrt(np.maximum((dataT * dataT).sum(axis=1), 1e-12))
    cn8 = [np.ascontiguousarray((dataT[b] * rn[b][None, :]).astype(F8))
           for b in range(B)]

    def pack_c8i(cn):
        # raw[p, d, ki, 2t+j] = cn[(2d+j)*128 + p, ki*128 + (127-t)]
        DPh, KBk = D // 256, N // 128
        C = np.asarray(cn).reshape(DPh, 2, 128, KBk, 128)
        C = C[..., ::-1]
        C = C.transpose(2, 0, 3, 4, 1)
        return np.ascontiguousarray(C.reshape(128, DPh * KBk * 256))
    rq_c = [np.ascontiguousarray(dataT[b].astype(F8 if MRAW8 else BF))
            for b in range(B)]

    Wt = _posenc(N, D) @ W_exp[1:] + W_exp[0]               # [N, M]
    KB = N // 128

    def pack_wt(W):
        if SWI_Z:
            # DoubleRowSwInterleave layout: per (kp, mj) a 256-col block
            # raw[p, 2t+j] = W[(2kp+j)*128 + p, mj*128 + (127-t)]
            C = W.reshape(KB // 2, 2, 128, M // 128, 128)  # [kp, j, p, mj, c]
            C = C[..., ::-1]                               # c -> t
            C = C.transpose(2, 0, 3, 4, 1)                 # [p, kp, mj, t, j]
            return np.ascontiguousarray(
                C.reshape(128, KB * M)).astype(F8)
        return np.ascontiguousarray(
            W.reshape(KB, 128, M).transpose(1, 0, 2).reshape(128, KB * M)
        ).astype(F8)

    # rotate key columns so this core's query rows are always keys [0:NQ];
    # Wt is rotated identically (the k-sum is permutation-invariant when
    # Wt rows follow their keys).
    wt8 = [pack_wt(Wt), pack_wt(np.roll(Wt, -NQ, axis=0))]
    cn8_rot = [np.ascontiguousarray(np.roll(a, -NQ, axis=1)) for a in cn8]
    c8i = [pack_c8i(a) for a in cn8] if SWI_G else [None] * B
    c8i_rot = [pack_c8i(a) for a in cn8_rot] if SWI_G else [None] * B
    wm1_c = W_merge[:D].astype(F8 if MRAW8 else BF)
    wm2_bf = W_merge[D:].astype(BF)
    bexp_r = np.ascontiguousarray(b_exp.reshape(M // 128, 128))

    in_maps = []
    for c in range(num_cores):
        b, h = c // 2, c % 2
        m = {
            "c8": cn8[b] if h == 0 else cn8_rot[b],
            "rq": np.ascontiguousarray(rq_c[b][:, h * NQ:(h + 1) * NQ]),
            "wt8": wt8[h],
            "wm1": wm1_c,
            "wm2": wm2_bf,
            "bexp": bexp_r,
        }
        if SWI_G:
            m["c8i"] = c8i[b] if h == 0 else c8i_rot[b]
        in_maps.append(m)
    return in_maps


_NC_CACHE = {}


def get_nc():
    if "full" not in _NC_CACHE:
        _NC_CACHE["full"] = build_nc()
    return _NC_CACHE["full"]


def kernel(data, W_exp, b_exp, W_merge):
    nc = get_nc()
    in_maps = make_in_maps(data, W_exp, b_exp, W_merge)
    res = run_bass_kernel_spmd(nc, in_maps, core_ids=list(range(NCORES)))
    out = np.empty((B, N, M), dtype=np.float32)
    for c in range(NCORES):
        b, h = c // 2, c % 2
        out[b, h * NQ:(h + 1) * NQ] = res.results[c]["out"].astype(np.float32)
    return out
